# revision 1
# baseline (speedup 1.0000x reference)
"""Causal multi-head self-attention (RoPE) Trainium2 kernel.

Model (from the reference nn.Module):
  D_MODEL=1024, NUM_HEADS=16, D_K=64, THETA=10000, BATCH=2, SEQ=2048.
  qkv = x @ w_qkv.T ; q,k get interleaved-pair RoPE; causal softmax(q k^T/8) v;
  out = attn_out @ w_o.T.

Sharding: tensor-parallel over heads. 8 cores x 2 heads each. x is
replicated (transposed on host), per-core w_qkv/w_o head slices. Each core
produces a partial y.T (full [1024, 4096]); host sums partials and
transposes back.

On-device layout is fully "transposed" (feature-on-partition, token-on-free):
  xT [1024, 4096], qT/kT [128, 4096] (2 heads x 64 dims on partitions),
  score tiles sT [k=128, q=512] for both heads side by side in one 2-bank
  PSUM tile, causal mask added on the PE (identity x (-30000) table matmul),
  one exp per k-chunk on ACT, PV against PE-transposed V with an appended
  ones column producing the softmax denominators in the same matmul,
  normalization by reciprocal + DMA partition-broadcast, final projection
  contracting the 128 on-core head-dims.

All matmul operands are float32r (TF32-class, full PE rate at N>=512).
"""

import math
import numpy as np
from contextlib import ExitStack

import concourse.bacc as bacc
import concourse.mybir as mybir
import concourse.tile as tile
from concourse.bass_utils import run_bass_kernel_spmd

f32 = mybir.dt.float32
f32r = mybir.dt.float32r
f16 = mybir.dt.float16

D = 1024          # d_model
H = 16            # total heads
DK = 64           # head dim
B = 2
S = 2048
T = B * S         # 4096 tokens
NCORES = 8
HPC = H // NCORES  # heads per core = 2
THETA = 10000.0
NEG = -30000.0     # causal-mask additive constant (exp underflows to 0)

TCH = 512          # token chunk (matmul N)
NTCH = T // TCH    # 8
KCH = 128          # key chunk (score-tile partitions)
DCH = 128          # d_model contraction chunk
NBLK = T // KCH    # 32

SWAP_MASK = [m ^ 1 for m in range(32)]  # adjacent-pair swap, per 32-quadrant

_PROGRAM = None


def _build_program():
    nc = bacc.Bacc("TRN2", target_bir_lowering=False, debug=False)

    xT = nc.dram_tensor("xT", [D, T], f16, kind="ExternalInput")
    wqkvT = nc.dram_tensor("wqkvT", [D, 3 * 128], f16, kind="ExternalInput")
    woT = nc.dram_tensor("woT", [128, D], f32r, kind="ExternalInput")
    crep = nc.dram_tensor("crep", [128, S], f16, kind="ExternalInput")
    ssign = nc.dram_tensor("ssign", [128, S], f16, kind="ExternalInput")
    maskneg = nc.dram_tensor("maskneg", [128, 896], f16, kind="ExternalInput")
    onesd = nc.dram_tensor("onesd", [128, 64], f32r, kind="ExternalInput")
    identr = nc.dram_tensor("identr", [128, 128], f16, kind="ExternalInput")
    yT = nc.dram_tensor("yT", [D, T], mybir.dt.bfloat16, kind="ExternalOutput")

    xT_r = xT.rearrange("(n p) t -> n p t", p=DCH)          # [8, 128, T]
    wq_r = wqkvT.rearrange("(n p) c -> p n c", p=DCH)       # [128, 8, 384]

    with tile.TileContext(nc) as tc:
        with ExitStack() as ctx:
            singles = ctx.enter_context(tc.tile_pool(name="singles", bufs=1))

            wq_sb = singles.tile([128, 8, 3 * 128], f16)
            crep_sb = singles.tile([128, S], f16)
            ssign_sb = singles.tile([128, S], f16)
            for h4 in range(4):
                sl = slice(h4 * (S // 4), (h4 + 1) * (S // 4))
                nc.gpsimd.dma_start(out=crep_sb[:, sl], in_=crep[:, sl])
                nc.gpsimd.dma_start(out=ssign_sb[:, sl], in_=ssign[:, sl])
            mask_sb = singles.tile([128, 896], f16)
            nc.gpsimd.dma_start(out=mask_sb, in_=maskneg[:, :])
            identr_sb = singles.tile([128, 128], f16)
            nc.gpsimd.dma_start(out=identr_sb, in_=identr[:, :])
            wo_sb = singles.tile([128, D], f32r)
            nc.gpsimd.dma_start(out=wo_sb, in_=woT[:, :])
            ones_sb = singles.tile([1, 64], f32r)
            nc.gpsimd.dma_start(out=ones_sb, in_=onesd[0:1, 0:64])

            qT = singles.tile([128, T], f32r)
            kT = singles.tile([128, T], f32r)
            # V in natural layout per 128-token block:
            # cols 0:64 = V_A, col 64 = ones, 65:129 = V_B, col 129 = ones.
            # Both heads' lhsT slices end with the ones column -> softmax
            # sums land in OT row 64, O in rows 0:64.
            vaug = singles.tile([128, NBLK, 130], f16)
            nc.gpsimd.dma_start(out=vaug[:, :, 64], in_=onesd[:, 0:NBLK])
            nc.gpsimd.dma_start(out=vaug[:, :, 129], in_=onesd[:, 32:32 + NBLK])
            ocatT = singles.tile([128, T], f32r)

            xpool = ctx.enter_context(tc.tile_pool(name="xc", bufs=3))
            rope = ctx.enter_context(tc.tile_pool(name="rope", bufs=3))
            eps_p = ctx.enter_context(tc.tile_pool(name="e", bufs=8))
            rp = ctx.enter_context(tc.tile_pool(name="r", bufs=4))
            yp = ctx.enter_context(tc.tile_pool(name="y", bufs=3))

            _mk_pools = {}

            def qkv_chunk(tch, xc=None):
                ps1 = _mk_pools["ps1"]
                pst = _mk_pools["pst"]
                t0 = tch * TCH
                s0 = t0 % S  # RoPE tables repeat per batch
                if xc is None:
                    xc = xpool.tile([128, 8, TCH], f16, tag="xc")
                    for dc in range(8):
                        nc.sync.dma_start(
                            out=xc[:, dc, :], in_=xT_r[dc, :, t0:t0 + TCH])
                for mb in range(3):  # q, k, v
                    if mb == 2:
                        # V directly in natural layout: x-chunk as the
                        # stationary operand, per 128-token block
                        for sub in range(TCH // KCH):  # 4 token blocks
                            blk = tch * 4 + sub
                            fo = sub * KCH
                            pv = pst.tile([128, KCH], f32, tag="pv")
                            for dc in range(8):
                                nc.tensor.matmul(
                                    pv, xc[:, dc, fo:fo + KCH],
                                    wq_sb[:, dc, 256:384],
                                    start=(dc == 0), stop=(dc == 7))
                            nc.scalar.activation(
                                out=vaug[:, blk, 0:64], in_=pv[:, 0:64],
                                func=mybir.ActivationFunctionType.Copy)
                            nc.vector.tensor_copy(
                                out=vaug[:, blk, 65:129], in_=pv[:, 64:128])
                        continue
                    ps = ps1.tile([128, TCH], f32, tag="qkvps")
                    for dc in range(8):
                        nc.tensor.matmul(
                            ps, wq_sb[:, dc, mb * 128:(mb + 1) * 128],
                            xc[:, dc, :],
                            start=(dc == 0), stop=(dc == 7))
                    if mb < 2:
                        dst = qT if mb == 0 else kT
                        sh = rope.tile([128, TCH], f32, tag="sh")
                        nc.vector.stream_shuffle(
                            out=sh, in_=ps, mask=SWAP_MASK)
                        tm1 = rope.tile([128, TCH], f32, tag="tm1")
                        nc.vector.tensor_tensor(
                            out=tm1, in0=ps, in1=crep_sb[:, s0:s0 + TCH],
                            op=mybir.AluOpType.mult)
                        tm2 = rope.tile([128, TCH], f32, tag="tm2")
                        nc.vector.tensor_tensor(
                            out=tm2, in0=sh, in1=ssign_sb[:, s0:s0 + TCH],
                            op=mybir.AluOpType.mult)
                        nc.vector.tensor_tensor(
                            out=dst[:, t0:t0 + TCH], in0=tm1, in1=tm2,
                            op=mybir.AluOpType.add)

            def attn_qi(b, qi):
                ps_s = _mk_pools["ss"]
                ps_ot = _mk_pools["ot"]
                toff = b * S
                boff = b * (S // KCH)
                q0 = toff + qi * TCH
                nkj = 4 * qi + 4
                otA = ps_ot.tile([65, TCH], f32, tag="ot")
                otB = ps_ot.tile([65, TCH], f32, tag="ot")
                for kj in range(nkj):
                    k0 = toff + kj * KCH
                    blk = boff + kj
                    # diagonal blocks: only columns [o, TCH) can be
                    # unmasked; skip the dead triangle region.
                    o = max(0, KCH * (kj - 4 * qi))
                    diag = kj >= 4 * qi
                    pAB = ps_s.tile([128, 2, TCH], f32, tag="sps")
                    nc.tensor.matmul(
                        pAB[:, 0, o:TCH], kT[0:64, k0:k0 + KCH],
                        qT[0:64, q0 + o:q0 + TCH],
                        start=True, stop=not diag, skip_group_check=True)
                    nc.tensor.matmul(
                        pAB[:, 1, o:TCH], kT[64:128, k0:k0 + KCH],
                        qT[64:128, q0 + o:q0 + TCH],
                        start=True, stop=not diag, skip_group_check=True)
                    if diag:  # additive causal mask via PE
                        msl = mask_sb[:, 384:896 - o]
                        nc.tensor.matmul(
                            pAB[:, 0, o:TCH], identr_sb, msl,
                            start=False, stop=True, skip_group_check=True)
                        nc.tensor.matmul(
                            pAB[:, 1, o:TCH], identr_sb, msl,
                            start=False, stop=True, skip_group_check=True)
                    eAB = eps_p.tile([128, 2, TCH], f16, tag="eT")
                    nc.scalar.activation(
                        out=eAB[:, :, o:TCH], in_=pAB[:, :, o:TCH],
                        func=mybir.ActivationFunctionType.Exp)
                    nc.tensor.matmul(
                        otA[:, o:TCH], vaug[:, blk, 0:65], eAB[:, 0, o:TCH],
                        start=(kj == 0), stop=(kj == nkj - 1),
                        skip_group_check=True)
                    nc.tensor.matmul(
                        otB[:, o:TCH], vaug[:, blk, 65:130], eAB[:, 1, o:TCH],
                        start=(kj == 0), stop=(kj == nkj - 1),
                        skip_group_check=True)
                # normalize: ocatT[:, q] = O_unnorm * (1/sums) broadcast.
                # DVE copies OT out of PSUM right away (frees the bank);
                # the rest runs SBUF-side on DVE/DMA/GPSIMD.
                for hi, otX in ((0, otA), (1, otB)):
                    ot_sb = rp.tile([65, TCH], f32, tag="otsb")
                    nc.vector.tensor_copy(out=ot_sb, in_=otX)
                    rX = rp.tile([1, TCH], f32r, tag="rr")
                    with nc.allow_low_precision(
                            reason="f32r softmax denominators"):
                        nc.vector.reciprocal(out=rX, in_=ot_sb[64:65, :])
                    bc_ps = ps_ot.tile([64, TCH], f32, tag="ot")
                    nc.tensor.matmul(bc_ps, ones_sb, rX,
                                     start=True, stop=True)
                    nc.vector.tensor_tensor(
                        out=ocatT[hi * 64:(hi + 1) * 64, q0:q0 + TCH],
                        in0=ot_sb[0:64, :], in1=bc_ps,
                        op=mybir.AluOpType.mult)

            def proj(b, half):
                ps_s = _mk_pools["ss"]
                toff = b * S
                if True:
                    h0 = toff + half * (S // 2)
                    for eb in range(8):  # output-embedding 128-blocks
                        pys = ps_s.tile([128, S // 2], f32, tag="sps")
                        for tq in range(2):
                            nc.tensor.matmul(
                                pys[:, tq * TCH:(tq + 1) * TCH],
                                wo_sb[:, eb * 128:(eb + 1) * 128],
                                ocatT[:, h0 + tq * TCH:h0 + (tq + 1) * TCH],
                                start=True, stop=True)
                        y_sb = yp.tile([128, S // 2], mybir.dt.bfloat16,
                                       tag="ysb")
                        if eb % 2 == 0:
                            nc.vector.tensor_copy(out=y_sb, in_=pys)
                        else:
                            nc.scalar.activation(
                                out=y_sb, in_=pys,
                                func=mybir.ActivationFunctionType.Copy)
                        nc.sync.dma_start(
                            out=yT[eb * 128:(eb + 1) * 128, h0:h0 + S // 2],
                            in_=y_sb)

            # ---- emission: QKV phase, then attention, then projection ---
            with ExitStack() as c1:
                ps1 = c1.enter_context(
                    tc.tile_pool(name="ps1", bufs=4, space="PSUM"))
                pst = c1.enter_context(
                    tc.tile_pool(name="pst", bufs=3, space="PSUM"))
                _mk_pools["ps1"] = ps1
                _mk_pools["pst"] = pst
                xc0 = xpool.tile([128, 8, TCH], f16, tag="xc")
                for dc in range(8):
                    nc.sync.dma_start(out=wq_sb[:, dc, :],
                                      in_=wq_r[:, dc, :])
                    nc.sync.dma_start(out=xc0[:, dc, :],
                                      in_=xT_r[dc, :, 0:TCH])
                for tch in range(NTCH):
                    qkv_chunk(tch, xc=xc0 if tch == 0 else None)
            with ExitStack() as c2:
                ps_s = c2.enter_context(
                    tc.tile_pool(name="ss", bufs=3, space="PSUM"))
                ps_ot = c2.enter_context(
                    tc.tile_pool(name="ot", bufs=2, space="PSUM"))
                _mk_pools["ss"] = ps_s
                _mk_pools["ot"] = ps_ot
                for qi in range(4):
                    attn_qi(0, qi)
                attn_qi(1, 0)
                proj(0, 0)
                proj(0, 1)
                attn_qi(1, 1)
                attn_qi(1, 2)
                attn_qi(1, 3)
                proj(1, 0)
                proj(1, 1)

    nc.compile()
    return nc


def _host_prep(x, token_positions, w_qkv, w_o):
    """Build per-core input maps."""
    x = np.asarray(x, dtype=np.float32)
    w_qkv = np.asarray(w_qkv, dtype=np.float32)
    w_o = np.asarray(w_o, dtype=np.float32)
    pos = np.asarray(token_positions).astype(np.float64)

    xT = np.ascontiguousarray(x.reshape(T, D).T).astype(np.float16)

    half = DK // 2
    inv_freq = THETA ** (-np.arange(half, dtype=np.float64) / half)  # [32]
    ang = pos[:, None] * inv_freq[None, :]          # [S, 32]
    cos = np.cos(ang).astype(np.float16)            # [S, 32]
    sin = np.sin(ang).astype(np.float16)

    # interleaved pair layout: partition p (within a head's 64) has freq p//2
    cos_rows = np.repeat(cos.T, 2, axis=0)          # [64, S]
    sin_rows = np.repeat(sin.T, 2, axis=0)
    sgn = np.where(np.arange(64) % 2 == 0, -1.0, 1.0).astype(np.float16)
    ssin_rows = sin_rows * sgn[:, None]
    crep = np.vstack([cos_rows, cos_rows])          # [128, 2048]
    ssign = np.vstack([ssin_rows, ssin_rows])

    jj = np.arange(896)[None, :]
    pp = np.arange(128)[:, None]
    maskneg = np.where(jj >= pp + 384, 0.0, NEG).astype(np.float16)

    onesd = np.ones((128, 64), dtype=np.float32)
    identr_np = np.eye(128, dtype=np.float16)

    scale = 1.0 / math.sqrt(DK)
    in_maps = []
    for c in range(NCORES):
        hA, hB = HPC * c, HPC * c + 1
        wq = np.empty((3 * 128, D), dtype=np.float32)
        wq[0:64] = w_qkv[hA * DK:(hA + 1) * DK] * scale
        wq[64:128] = w_qkv[hB * DK:(hB + 1) * DK] * scale
        wq[128:192] = w_qkv[D + hA * DK:D + (hA + 1) * DK]
        wq[192:256] = w_qkv[D + hB * DK:D + (hB + 1) * DK]
        wq[256:320] = w_qkv[2 * D + hA * DK:2 * D + (hA + 1) * DK]
        wq[320:384] = w_qkv[2 * D + hB * DK:2 * D + (hB + 1) * DK]
        wqkvT = np.ascontiguousarray(wq.T).astype(np.float16)

        woTc = np.ascontiguousarray(
            w_o[:, hA * DK:(hB + 1) * DK].T)        # [128, 1024]

        in_maps.append({
            "xT": xT, "wqkvT": wqkvT, "woT": woTc,
            "crep": crep, "ssign": ssign, "maskneg": maskneg,
            "onesd": onesd, "identr": identr_np,
        })
    return in_maps


def _get_program():
    global _PROGRAM
    if _PROGRAM is None:
        _PROGRAM = _build_program()
    return _PROGRAM


def run_sharded(in_maps, **kwargs):
    nc = _get_program()
    return run_bass_kernel_spmd(nc, in_maps, core_ids=list(range(NCORES)),
                                **kwargs)


def kernel(x, token_positions, w_qkv, w_o):
    in_maps = _host_prep(x, token_positions, w_qkv, w_o)
    res = run_sharded(in_maps)
    acc = np.zeros((D, T), dtype=np.float64)
    for c in range(NCORES):
        acc += res.results[c]["yT"].astype(np.float32)
    y = acc.T.astype(np.float32).reshape(B, S, D)
    return y



# revision 4
# speedup vs baseline: 1.1364x; 1.1364x over previous
"""Causal multi-head self-attention (RoPE) Trainium2 kernel.

Model (from the reference nn.Module):
  D_MODEL=1024, NUM_HEADS=16, D_K=64, THETA=10000, BATCH=2, SEQ=2048.
  qkv = x @ w_qkv.T ; q,k get interleaved-pair RoPE; causal softmax(q k^T/8) v;
  out = attn_out @ w_o.T.

Sharding: tensor-parallel over heads. 8 cores x 2 heads each. x is
replicated (transposed on host), per-core w_qkv/w_o head slices. Each core
produces a partial y.T (full [1024, 4096]); host sums partials and
transposes back.

On-device layout is fully "transposed" (feature-on-partition, token-on-free):
  xT [1024, 4096], qT/kT [128, 4096] f16 (2 heads x 64 dims on partitions),
  score tiles sT [k=128, q=512] for both heads side by side in one 2-bank
  PSUM tile, causal masking via a 0/1 multiply on the exp output (DVE),
  one exp per k-chunk on ACT, PV against natural-layout V with an appended
  ones column producing the softmax denominators in the same matmul,
  normalization by reciprocal + partition-broadcast multiply, final
  projection contracting the 128 on-core head-dims.

All matmul operands are float16 (full PE rate). Emission interleaves the
QKV, attention and projection streams so PE never drains while ACT (exp)
and DVE (RoPE/normalize) pipeline underneath.
"""

import math
import numpy as np
from contextlib import ExitStack

import concourse.bacc as bacc
import concourse.mybir as mybir
import concourse.tile as tile
from concourse.bass_utils import run_bass_kernel_spmd

f32 = mybir.dt.float32
f32r = mybir.dt.float32r
f16 = mybir.dt.float16

D = 1024          # d_model
H = 16            # total heads
DK = 64           # head dim
B = 2
S = 2048
T = B * S         # 4096 tokens
NCORES = 8
HPC = H // NCORES  # heads per core = 2
THETA = 10000.0

TCH = 512          # token chunk (matmul N)
NTCH = T // TCH    # 8
KCH = 128          # key chunk (score-tile partitions)
NBLK = T // KCH    # 32

SWAP_MASK = [m ^ 1 for m in range(32)]  # adjacent-pair swap, per 32-quadrant

_PROGRAM = None

MULT = mybir.AluOpType.mult
ADD = mybir.AluOpType.add


def _build_program():
    nc = bacc.Bacc("TRN2", target_bir_lowering=False, debug=False)

    xT = nc.dram_tensor("xT", [D, T], f16, kind="ExternalInput")
    wqkvT = nc.dram_tensor("wqkvT", [D, 3 * 128], f16, kind="ExternalInput")
    woT = nc.dram_tensor("woT", [128, D], f16, kind="ExternalInput")
    crep = nc.dram_tensor("crep", [128, S], f16, kind="ExternalInput")
    ssign = nc.dram_tensor("ssign", [128, S], f16, kind="ExternalInput")
    mask01 = nc.dram_tensor("mask01", [128, 2, KCH], f16, kind="ExternalInput")
    yT = nc.dram_tensor("yT", [D, T], f16, kind="ExternalOutput")

    xT_r = xT.rearrange("(n p) t -> n p t", p=128)      # [8, 128, T]
    xT_p = xT.rearrange("(n p) t -> p n t", p=128)      # [128, 8, T]
    wq_r = wqkvT.rearrange("(n p) c -> p n c", p=128)   # [128, 8, 384]

    with tile.TileContext(nc) as tc:
        with ExitStack() as ctx:
            singles = ctx.enter_context(tc.tile_pool(name="singles", bufs=1))

            wq_sb = singles.tile([128, 8, 3 * 128], f16)
            crep_sb = singles.tile([128, S], f16)
            ssign_sb = singles.tile([128, S], f16)
            mask_sb = singles.tile([128, 2, KCH], f16)
            wo_sb = singles.tile([128, D], f16)

            qT = singles.tile([128, T], f16)
            kT = singles.tile([128, T], f16)
            # V in natural layout per 128-token block:
            # cols 0:64 = V_A, col 64 = ones, 65:129 = V_B, col 129 = ones.
            # Both heads' lhsT slices end with the ones column -> softmax
            # sums land in OT row 64, O in rows 0:64.
            vaug = singles.tile([128, NBLK, 130], f16)
            ocatT = singles.tile([128, T], f16)

            # --- singles loads (SWDGE via gpsimd; keeps HWDGE free) -----
            nc.gpsimd.dma_start(out=wq_sb, in_=wq_r[:, :, :])
            nc.gpsimd.dma_start(out=crep_sb, in_=crep[:, :])
            nc.gpsimd.dma_start(out=ssign_sb, in_=ssign[:, :])
            nc.gpsimd.dma_start(out=mask_sb, in_=mask01[:, :, :])
            nc.gpsimd.dma_start(out=wo_sb, in_=woT[:, :])
            nc.gpsimd.memset(vaug[:, :, 64], 1.0)
            nc.gpsimd.memset(vaug[:, :, 129], 1.0)

            xpool = ctx.enter_context(tc.tile_pool(name="xc", bufs=3))
            rope_p = ctx.enter_context(tc.tile_pool(name="rope", bufs=2))
            eps_p = ctx.enter_context(tc.tile_pool(name="e", bufs=4))
            rp = ctx.enter_context(tc.tile_pool(name="r", bufs=2))
            yp = ctx.enter_context(tc.tile_pool(name="y", bufs=4))

            pp = ctx.enter_context(
                tc.tile_pool(name="pp", bufs=2, space="PSUM"))
            ss = ctx.enter_context(
                tc.tile_pool(name="ss", bufs=2, space="PSUM"))
            otp = ctx.enter_context(
                tc.tile_pool(name="ot", bufs=1, space="PSUM"))

            def rope(ps, dst, t0, s0):
                # StreamShuffle requires src/dst dtype match -> f32 out
                sh = rope_p.tile([128, TCH], f32, tag="sh")
                nc.vector.stream_shuffle(out=sh, in_=ps, mask=SWAP_MASK)
                tm1 = rope_p.tile([128, TCH], f16, tag="tm1")
                nc.vector.tensor_tensor(
                    out=tm1, in0=ps, in1=crep_sb[:, s0:s0 + TCH], op=MULT)
                tm2 = rope_p.tile([128, TCH], f16, tag="tm2")
                nc.vector.tensor_tensor(
                    out=tm2, in0=sh, in1=ssign_sb[:, s0:s0 + TCH], op=MULT)
                nc.vector.tensor_tensor(
                    out=dst[:, t0:t0 + TCH], in0=tm1, in1=tm2, op=ADD)

            def qkv_chunk(tch):
                t0 = tch * TCH
                s0 = t0 % S  # RoPE tables repeat per batch
                xc = xpool.tile([128, 8, TCH], f16, tag="xc")
                if tch < 2:  # per-dc loads: first matmuls start sooner
                    for dc in range(8):
                        nc.sync.dma_start(
                            out=xc[:, dc, :], in_=xT_r[dc, :, t0:t0 + TCH])
                else:
                    nc.sync.dma_start(
                        out=xc, in_=xT_p[:, :, t0:t0 + TCH])
                for mb in range(2):  # q, k
                    ps = pp.tile([128, TCH], f32, tag="pp")
                    for dc in range(8):
                        nc.tensor.matmul(
                            ps, wq_sb[:, dc, mb * 128:(mb + 1) * 128],
                            xc[:, dc, :],
                            start=(dc == 0), stop=(dc == 7))
                    rope(ps, qT if mb == 0 else kT, t0, s0)
                # V directly in natural layout: x-chunk as the stationary
                # operand, per 128-token block; 4 blocks share one PSUM tile
                pv = pp.tile([128, 4, KCH], f32, tag="pp")
                for sub in range(4):
                    for dc in range(8):
                        nc.tensor.matmul(
                            pv[:, sub, :],
                            xc[:, dc, sub * KCH:(sub + 1) * KCH],
                            wq_sb[:, dc, 256:384],
                            start=(dc == 0), stop=(dc == 7))
                blk0 = tch * 4
                nc.vector.tensor_copy(
                    out=vaug[:, blk0:blk0 + 4, 0:64], in_=pv[:, :, 0:64])
                nc.vector.tensor_copy(
                    out=vaug[:, blk0:blk0 + 4, 65:129], in_=pv[:, :, 64:128])

            # ---- projection, emitted as independent per-eb units --------
            def proj_unit(b, qi, eb):
                h0 = b * S + qi * TCH
                pys = ss.tile([128, 2, TCH], f32, tag="ss")
                nc.tensor.matmul(
                    pys[:, 0, :], wo_sb[:, eb * 128:(eb + 1) * 128],
                    ocatT[:, h0:h0 + TCH], start=True, stop=True)
                y_sb = yp.tile([128, TCH], f16, tag="ysb")
                if eb % 2 == 0:
                    nc.vector.tensor_copy(out=y_sb, in_=pys[:, 0, :])
                else:
                    nc.scalar.activation(
                        out=y_sb, in_=pys[:, 0, :],
                        func=mybir.ActivationFunctionType.Copy)
                nc.sync.dma_start(
                    out=yT[eb * 128:(eb + 1) * 128, h0:h0 + TCH], in_=y_sb)

            proj_ready = []   # (b, qi, eb) units whose ocatT deps are done

            def filler():
                if proj_ready:
                    proj_unit(*proj_ready.pop(0))

            def attn_qi(b, qi):
                toff = b * S
                boff = b * (S // KCH)
                q0 = toff + qi * TCH
                nkj = 4 * qi + 4
                ot = otp.tile([65, 2, TCH], f32, tag="ot")
                pend = None
                for kj in range(nkj):
                    k0 = toff + kj * KCH
                    blk = boff + kj
                    # diagonal blocks: only columns [o, TCH) can be
                    # unmasked; skip the dead triangle region.
                    o = max(0, KCH * (kj - 4 * qi))
                    diag = kj >= 4 * qi
                    pAB = ss.tile([128, 2, TCH], f32, tag="ss")
                    for hi in range(2):
                        nc.tensor.matmul(
                            pAB[:, hi, o:TCH],
                            kT[hi * 64:(hi + 1) * 64, k0:k0 + KCH],
                            qT[hi * 64:(hi + 1) * 64, q0 + o:q0 + TCH],
                            start=True, stop=True, skip_group_check=True)
                    eAB = eps_p.tile([128, 2, TCH], f16, tag="eT")
                    nc.scalar.activation(
                        out=eAB[:, :, o:TCH], in_=pAB[:, :, o:TCH],
                        func=mybir.ActivationFunctionType.Exp)
                    if diag:  # zero the in-block upper triangle
                        nc.vector.tensor_tensor(
                            out=eAB[:, :, o:o + KCH],
                            in0=eAB[:, :, o:o + KCH],
                            in1=mask_sb, op=MULT)
                    # software pipeline: PV for the previous block runs
                    # while this block's exp drains on ACT
                    if pend is not None:
                        _emit_pv(ot, nkj, *pend)
                    pend = (kj, eAB, o, blk)
                    filler()
                _emit_pv(ot, nkj, *pend)
                # normalize: ocatT[:, q] = O_unnorm * (1/sums) broadcast
                rX = rp.tile([1, 2, TCH], f32r, tag="rr")
                with nc.allow_low_precision(
                        reason="f32r softmax denominators"):
                    nc.vector.reciprocal(out=rX, in_=ot[64:65, :, :])
                bc = rp.tile([64, 2, TCH], f32r, tag="bc")
                nc.gpsimd.partition_broadcast(bc, rX, channels=64)
                for hi in range(2):
                    nc.vector.tensor_tensor(
                        out=ocatT[hi * 64:(hi + 1) * 64, q0:q0 + TCH],
                        in0=ot[0:64, hi, :],
                        in1=bc[:, hi, :],
                        op=MULT)

            def _emit_pv(ot, nkj, kj, eAB, o, blk):
                for hi in range(2):
                    nc.tensor.matmul(
                        ot[:, hi, o:TCH],
                        vaug[:, blk, 65 * hi:65 * hi + 65],
                        eAB[:, hi, o:TCH],
                        start=(kj == 0), stop=(kj == nkj - 1),
                        skip_group_check=True)

            # ---- interleaved emission -----------------------------------
            # attn(b, qi) needs qkv chunks: b=0 -> 0..qi, b=1 -> 4..4+qi.
            # proj(b, qi, *) needs attn(b, qi) only.
            qkv_chunk(0)
            attn_qi(0, 0)
            proj_ready += [(0, 0, eb) for eb in range(8)]
            qkv_chunk(1)
            attn_qi(0, 1)
            proj_ready += [(0, 1, eb) for eb in range(8)]
            qkv_chunk(2)
            attn_qi(0, 2)
            proj_ready += [(0, 2, eb) for eb in range(8)]
            qkv_chunk(3)
            attn_qi(0, 3)
            proj_ready += [(0, 3, eb) for eb in range(8)]
            qkv_chunk(4)
            attn_qi(1, 0)
            proj_ready += [(1, 0, eb) for eb in range(8)]
            qkv_chunk(5)
            attn_qi(1, 1)
            proj_ready += [(1, 1, eb) for eb in range(8)]
            qkv_chunk(6)
            attn_qi(1, 2)
            proj_ready += [(1, 2, eb) for eb in range(8)]
            qkv_chunk(7)
            attn_qi(1, 3)
            proj_ready += [(1, 3, eb) for eb in range(8)]
            while proj_ready:
                proj_unit(*proj_ready.pop(0))

    nc.compile()
    return nc


def _host_prep(x, token_positions, w_qkv, w_o):
    """Build per-core input maps."""
    x = np.asarray(x, dtype=np.float32)
    w_qkv = np.asarray(w_qkv, dtype=np.float32)
    w_o = np.asarray(w_o, dtype=np.float32)
    pos = np.asarray(token_positions).astype(np.float64)

    xT = np.ascontiguousarray(x.reshape(T, D).T).astype(np.float16)

    half = DK // 2
    inv_freq = THETA ** (-np.arange(half, dtype=np.float64) / half)  # [32]
    ang = pos[:, None] * inv_freq[None, :]          # [S, 32]
    cos = np.cos(ang).astype(np.float16)            # [S, 32]
    sin = np.sin(ang).astype(np.float16)

    # interleaved pair layout: partition p (within a head's 64) has freq p//2
    cos_rows = np.repeat(cos.T, 2, axis=0)          # [64, S]
    sin_rows = np.repeat(sin.T, 2, axis=0)
    sgn = np.where(np.arange(64) % 2 == 0, -1.0, 1.0).astype(np.float16)
    ssin_rows = sin_rows * sgn[:, None]
    crep = np.vstack([cos_rows, cos_rows])          # [128, 2048]
    ssign = np.vstack([ssin_rows, ssin_rows])

    # 0/1 triangle for the diagonal 128-col slab of each diag block:
    # column cc (local) valid iff cc >= p (key partition index)
    cc = np.arange(KCH)[None, :]
    ppi = np.arange(KCH)[:, None]
    tri = (cc >= ppi).astype(np.float16)            # [128, 128]
    mask01 = np.ascontiguousarray(
        np.broadcast_to(tri[:, None, :], (KCH, 2, KCH)))

    scale = 1.0 / math.sqrt(DK)
    in_maps = []
    for c in range(NCORES):
        hA, hB = HPC * c, HPC * c + 1
        wq = np.empty((3 * 128, D), dtype=np.float32)
        wq[0:64] = w_qkv[hA * DK:(hA + 1) * DK] * scale
        wq[64:128] = w_qkv[hB * DK:(hB + 1) * DK] * scale
        wq[128:192] = w_qkv[D + hA * DK:D + (hA + 1) * DK]
        wq[192:256] = w_qkv[D + hB * DK:D + (hB + 1) * DK]
        wq[256:320] = w_qkv[2 * D + hA * DK:2 * D + (hA + 1) * DK]
        wq[320:384] = w_qkv[2 * D + hB * DK:2 * D + (hB + 1) * DK]
        wqkvT = np.ascontiguousarray(wq.T).astype(np.float16)

        woTc = np.ascontiguousarray(
            w_o[:, hA * DK:(hB + 1) * DK].T).astype(np.float16)  # [128, 1024]

        in_maps.append({
            "xT": xT, "wqkvT": wqkvT, "woT": woTc,
            "crep": crep, "ssign": ssign, "mask01": mask01,
        })
    return in_maps


def _get_program():
    global _PROGRAM
    if _PROGRAM is None:
        _PROGRAM = _build_program()
    return _PROGRAM


def run_sharded(in_maps, **kwargs):
    nc = _get_program()
    return run_bass_kernel_spmd(nc, in_maps, core_ids=list(range(NCORES)),
                                **kwargs)


def kernel(x, token_positions, w_qkv, w_o):
    in_maps = _host_prep(x, token_positions, w_qkv, w_o)
    res = run_sharded(in_maps)
    acc = np.zeros((D, T), dtype=np.float64)
    for c in range(NCORES):
        acc += res.results[c]["yT"].astype(np.float32)
    y = acc.T.astype(np.float32).reshape(B, S, D)
    return y


# revision 25
# speedup vs baseline: 1.2737x; 1.1208x over previous
"""Causal multi-head self-attention (RoPE) Trainium2 kernel.

Model (from the reference nn.Module):
  D_MODEL=1024, NUM_HEADS=16, D_K=64, THETA=10000, BATCH=2, SEQ=2048.
  qkv = x @ w_qkv.T ; q,k get interleaved-pair RoPE; causal softmax(q k^T/8) v;
  out = attn_out @ w_o.T.

Sharding: tensor-parallel over heads. 8 cores x 2 heads each. x is
replicated (transposed on host), per-core w_qkv/w_o head slices. Each core
produces a partial y.T (full [1024, 4096]); host sums partials and
transposes back.

On-device layout is fully "transposed" (feature-on-partition, token-on-free):
  xT [1024, 4096], qT/kT [128, 4096] f16 (2 heads x 64 dims on partitions),
  score tiles sT [k=128, q=512] for both heads side by side in one 2-bank
  PSUM tile, causal masking via a 0/1 multiply on the exp output (Pool),
  one exp per k-chunk on ACT, PV against natural-layout V with an appended
  ones column producing the softmax denominators in the same matmul,
  normalization by reciprocal + partition-broadcast multiply (deferred out
  of the critical path), final projection contracting the 128 on-core
  head-dims.

All matmul operands are float16 (full PE rate). Emission interleaves the
QKV, attention and projection streams so PE never drains while ACT (exp)
and DVE (RoPE/normalize/output copies) pipeline underneath; Pool (gpsimd)
takes the SBUF-only elementwise work.
"""

import math
import numpy as np
from contextlib import ExitStack

import concourse.bacc as bacc
import concourse.mybir as mybir
import concourse.tile as tile
from concourse.bass_utils import run_bass_kernel_spmd

f32 = mybir.dt.float32
f32r = mybir.dt.float32r
f16 = mybir.dt.float16

D = 1024          # d_model
H = 16            # total heads
DK = 64           # head dim
B = 2
S = 2048
T = B * S         # 4096 tokens
NCORES = 8
HPC = H // NCORES  # heads per core = 2
THETA = 10000.0

TCH = 512          # token chunk (matmul N)
NTCH = T // TCH    # 8
KCH = 128          # key chunk (score-tile partitions)
NBLK = T // KCH    # 32

SWAP_MASK = [m ^ 1 for m in range(32)]  # adjacent-pair swap, per 32-quadrant

PV_DEPTH = 4       # software-pipeline depth for score->exp->PV
BUFS_X = 3         # x-chunk double buffering
BUFS_ROPE = 2
BUFS_E = 6
BUFS_Y = 4
Y_SPLIT = 0        # y copies: 0 = all on DVE (ACT stays exp-only)

_PROGRAM = None

MULT = mybir.AluOpType.mult
ADD = mybir.AluOpType.add


def _build_program():
    nc = bacc.Bacc("TRN2", target_bir_lowering=False, debug=False)

    xT = nc.dram_tensor("xT", [D, T], f16, kind="ExternalInput")
    wqkvT = nc.dram_tensor("wqkvT", [D, 3 * 128], f16, kind="ExternalInput")
    woT = nc.dram_tensor("woT", [128, D], f16, kind="ExternalInput")
    crep = nc.dram_tensor("crep", [128, S], f16, kind="ExternalInput")
    ssign = nc.dram_tensor("ssign", [128, S], f16, kind="ExternalInput")
    mask01 = nc.dram_tensor("mask01", [128, 2, KCH], f16, kind="ExternalInput")
    yT = nc.dram_tensor("yT", [D, T], f16, kind="ExternalOutput")

    xT_r = xT.rearrange("(n p) t -> n p t", p=128)      # [8, 128, T]
    xT_p = xT.rearrange("(n p) t -> p n t", p=128)      # [128, 8, T]
    wq_r = wqkvT.rearrange("(n p) c -> p n c", p=128)   # [128, 8, 384]

    with tile.TileContext(nc) as tc:
        with ExitStack() as ctx:
            singles = ctx.enter_context(tc.tile_pool(name="singles", bufs=1))

            wq_sb = singles.tile([128, 8, 3 * 128], f16)
            crep_sb = singles.tile([128, S], f16)
            ssign_sb = singles.tile([128, S], f16)
            mask_sb = singles.tile([128, 2, KCH], f16)
            wo_sb = singles.tile([128, D], f16)

            qT = singles.tile([128, T], f16)
            kT = singles.tile([128, T], f16)
            # V in natural layout per 128-token block:
            # cols 0:64 = V_A, col 64 = ones, 65:129 = V_B, col 129 = ones.
            # Both heads' lhsT slices end with the ones column -> softmax
            # sums land in OT row 64, O in rows 0:64.
            vaug = singles.tile([128, NBLK, 130], f16)
            ocatT = singles.tile([128, T], f16)

            xpool = ctx.enter_context(tc.tile_pool(name="xc", bufs=BUFS_X))

            # --- singles loads, ordered for fastest rope/matmul start ----
            # (wq-q then the first x chunk on sync/HWDGE; tables via SWDGE)
            nc.sync.dma_start(out=wq_sb[:, :, 0:128], in_=wq_r[:, :, 0:128])
            xc0 = xpool.tile([128, 8, TCH], f16, tag="xc")
            nc.sync.dma_start(out=xc0[:, 0:4, :], in_=xT_p[:, 0:4, 0:TCH])
            nc.sync.dma_start(out=xc0[:, 4:8, :], in_=xT_p[:, 4:8, 0:TCH])
            nc.sync.dma_start(out=wq_sb[:, :, 128:256],
                              in_=wq_r[:, :, 128:256])
            nc.gpsimd.dma_start(out=crep_sb[:, 0:TCH], in_=crep[:, 0:TCH])
            nc.gpsimd.dma_start(out=ssign_sb[:, 0:TCH], in_=ssign[:, 0:TCH])
            nc.sync.dma_start(out=wq_sb[:, :, 256:384],
                              in_=wq_r[:, :, 256:384])
            nc.gpsimd.dma_start(out=crep_sb[:, TCH:S], in_=crep[:, TCH:S])
            nc.gpsimd.dma_start(out=ssign_sb[:, TCH:S], in_=ssign[:, TCH:S])
            nc.gpsimd.dma_start(out=mask_sb, in_=mask01[:, :, :])
            nc.gpsimd.dma_start(out=wo_sb, in_=woT[:, :])
            nc.gpsimd.memset(vaug[:, :, 64], 1.0)
            nc.gpsimd.memset(vaug[:, :, 129], 1.0)
            rope_p = ctx.enter_context(tc.tile_pool(name="rope", bufs=BUFS_ROPE))
            eps_p = ctx.enter_context(tc.tile_pool(name="e", bufs=BUFS_E))
            rp = ctx.enter_context(tc.tile_pool(name="r", bufs=2))
            yp = ctx.enter_context(tc.tile_pool(name="y", bufs=BUFS_Y))

            pp = ctx.enter_context(
                tc.tile_pool(name="pp", bufs=2, space="PSUM"))
            ss = ctx.enter_context(
                tc.tile_pool(name="ss", bufs=2, space="PSUM"))
            otp = ctx.enter_context(
                tc.tile_pool(name="ot", bufs=1, space="PSUM"))

            def rope_front(ps, s0):
                """DVE shuffle + cos-mult, Pool sin-mult; returns the two
                products to be summed by rope_back."""
                # StreamShuffle requires src/dst dtype match -> f32 out
                sh = rope_p.tile([128, TCH], f32, tag="sh")
                nc.vector.stream_shuffle(out=sh, in_=ps, mask=SWAP_MASK)
                tm1 = rope_p.tile([128, TCH], f16, tag="tm1")
                nc.vector.tensor_tensor(
                    out=tm1, in0=ps, in1=crep_sb[:, s0:s0 + TCH], op=MULT)
                tm2 = rope_p.tile([128, TCH], f16, tag="tm2")
                nc.gpsimd.tensor_tensor(
                    out=tm2, in0=sh, in1=ssign_sb[:, s0:s0 + TCH], op=MULT)
                return tm1, tm2

            def rope_back(dst, t0, tm1, tm2):
                nc.vector.tensor_tensor(
                    out=dst[:, t0:t0 + TCH], in0=tm1, in1=tm2, op=ADD)

            def qkv_chunk(tch):
                t0 = tch * TCH
                s0 = t0 % S  # RoPE tables repeat per batch
                if tch == 0:
                    xc = xc0  # preloaded with the singles
                else:
                    xc = xpool.tile([128, 8, TCH], f16, tag="xc")
                    if tch == 1:  # split load: first matmuls start sooner
                        nc.sync.dma_start(
                            out=xc[:, 0:4, :], in_=xT_p[:, 0:4, t0:t0 + TCH])
                        nc.sync.dma_start(
                            out=xc[:, 4:8, :], in_=xT_p[:, 4:8, t0:t0 + TCH])
                    else:
                        nc.sync.dma_start(
                            out=xc, in_=xT_p[:, :, t0:t0 + TCH])
                halves = []
                for mb in range(2):  # q, k
                    ps = pp.tile([128, TCH], f32, tag="pp")
                    for dc in range(8):
                        nc.tensor.matmul(
                            ps, wq_sb[:, dc, mb * 128:(mb + 1) * 128],
                            xc[:, dc, :],
                            start=(dc == 0), stop=(dc == 7))
                    halves.append(rope_front(ps, s0))
                # the adds go last: the DVE queue never parks at its head
                # waiting for Pool's sin-mult of the same chunk
                rope_back(qT, t0, *halves[0])
                rope_back(kT, t0, *halves[1])
                # V directly in natural layout: x-chunk as the stationary
                # operand, per 128-token block; 4 blocks share one PSUM tile
                pv = pp.tile([128, 4, KCH], f32, tag="pp")
                for sub in range(4):
                    for dc in range(8):
                        nc.tensor.matmul(
                            pv[:, sub, :],
                            xc[:, dc, sub * KCH:(sub + 1) * KCH],
                            wq_sb[:, dc, 256:384],
                            start=(dc == 0), stop=(dc == 7))
                blk0 = tch * 4
                nc.scalar.activation(
                    out=vaug[:, blk0:blk0 + 4, 0:64], in_=pv[:, :, 0:64],
                    func=mybir.ActivationFunctionType.Copy)
                nc.scalar.activation(
                    out=vaug[:, blk0:blk0 + 4, 65:129], in_=pv[:, :, 64:128],
                    func=mybir.ActivationFunctionType.Copy)

            # ---- projection, emitted as independent per-eb units --------
            def proj_unit(b, qi, eb, tail=False):
                h0 = b * S + qi * TCH
                pys = ss.tile([128, 2, TCH], f32, tag="ss")
                nc.tensor.matmul(
                    pys[:, 0, :], wo_sb[:, eb * 128:(eb + 1) * 128],
                    ocatT[:, h0:h0 + TCH], start=True, stop=True)
                y_sb = yp.tile([128, TCH], f16, tag="ysb")
                if Y_SPLIT and eb % Y_SPLIT == 0:  # share copies with ACT
                    nc.scalar.activation(
                        out=y_sb, in_=pys[:, 0, :],
                        func=mybir.ActivationFunctionType.Copy)
                else:
                    nc.vector.tensor_copy(out=y_sb, in_=pys[:, 0, :])
                nc.sync.dma_start(
                    out=yT[eb * 128:(eb + 1) * 128, h0:h0 + TCH], in_=y_sb)

            # deferred work queues: normalize closures run a few PE-ops into
            # the following segment so their recip/broadcast latency hides.
            pending = []      # normalize closures
            proj_ready = []   # (b, qi, eb) units whose ocatT deps are done

            def filler():
                if pending:
                    pending.pop(0)()
                elif proj_ready:
                    proj_unit(*proj_ready.pop(0))

            def flush(tail=False):
                while pending:
                    pending.pop(0)()
                while proj_ready:
                    proj_unit(*proj_ready.pop(0), tail=tail)

            def attn_qi(b, qi, last=False):
                toff = b * S
                boff = b * (S // KCH)
                q0 = toff + qi * TCH
                nkj = 4 * qi + 4
                ot = otp.tile([65, 2, TCH], f32, tag="ot")
                pend = []  # 2-deep software pipeline: PV(kj) runs after
                # sc(kj+2), giving exp(kj) two score-stretches to drain
                for kj in range(nkj):
                    k0 = toff + kj * KCH
                    blk = boff + kj
                    # diagonal blocks: only columns [o, TCH) can be
                    # unmasked; skip the dead triangle region.
                    o = max(0, KCH * (kj - 4 * qi))
                    diag = kj >= 4 * qi
                    pAB = ss.tile([128, 2, TCH], f32, tag="ss")
                    for hi in range(2):
                        nc.tensor.matmul(
                            pAB[:, hi, o:TCH],
                            kT[hi * 64:(hi + 1) * 64, k0:k0 + KCH],
                            qT[hi * 64:(hi + 1) * 64, q0 + o:q0 + TCH],
                            start=True, stop=True, skip_group_check=True)
                    eAB = eps_p.tile([128, 2, TCH], f16, tag="eT")
                    nc.scalar.activation(
                        out=eAB[:, :, o:TCH], in_=pAB[:, :, o:TCH],
                        func=mybir.ActivationFunctionType.Exp)
                    if diag:  # zero the in-block upper triangle (f16 4x DVE)
                        nc.vector.tensor_tensor(
                            out=eAB[:, :, o:o + KCH],
                            in0=eAB[:, :, o:o + KCH],
                            in1=mask_sb, op=MULT)
                    if len(pend) >= PV_DEPTH:
                        _emit_pv(ot, nkj, *pend.pop(0))
                    pend.append((kj, eAB, o, blk))
                    filler()

                def tail():
                    # drain the pipeline + reciprocal; runs after the NEXT
                    # segment's matmuls so the last exps never stall PE
                    for p in pend:
                        _emit_pv(ot, nkj, *p)
                    rX = rp.tile([1, 2, TCH], f32r, tag="rr")
                    with nc.allow_low_precision(
                            reason="f32r softmax denominators"):
                        nc.vector.reciprocal(out=rX, in_=ot[64:65, :, :])

                    def _norm():
                        # TensorTensor may read at most ONE PSUM operand
                        # (ot), so the broadcast reciprocal must be SBUF
                        bc = rp.tile([64, 2, TCH], f32r, tag="bc")
                        nc.gpsimd.partition_broadcast(bc, rX, channels=64)
                        for hi in range(2):
                            nc.vector.tensor_tensor(
                                out=ocatT[hi * 64:(hi + 1) * 64,
                                          q0:q0 + TCH],
                                in0=ot[0:64, hi, :],
                                in1=bc[:, hi, :],
                                op=MULT)

                    pending.append(_norm)
                    proj_ready.extend((b, qi, eb) for eb in range(8))

                return tail

            def _emit_pv(ot, nkj, kj, eAB, o, blk):
                for hi in range(2):
                    nc.tensor.matmul(
                        ot[:, hi, o:TCH],
                        vaug[:, blk, 65 * hi:65 * hi + 65],
                        eAB[:, hi, o:TCH],
                        start=(kj == 0), stop=(kj == nkj - 1),
                        skip_group_check=True)

            # ---- interleaved emission -----------------------------------
            # attn(b, qi) needs qkv chunks: b=0 -> 0..qi, b=1 -> 4..4+qi.
            # proj(b, qi, *) needs attn(b, qi)'s normalize only.
            prev_tail = None
            for step in range(NTCH):
                b, qi = divmod(step, 4)
                qkv_chunk(step)
                if prev_tail is not None:
                    prev_tail()
                prev_tail = attn_qi(b, qi, last=(step == NTCH - 1))
            prev_tail()
            flush(tail=True)

    nc.compile()
    return nc


def _host_prep(x, token_positions, w_qkv, w_o):
    """Build per-core input maps."""
    x = np.asarray(x, dtype=np.float32)
    w_qkv = np.asarray(w_qkv, dtype=np.float32)
    w_o = np.asarray(w_o, dtype=np.float32)
    pos = np.asarray(token_positions).astype(np.float64)

    xT = np.ascontiguousarray(x.reshape(T, D).T).astype(np.float16)

    half = DK // 2
    inv_freq = THETA ** (-np.arange(half, dtype=np.float64) / half)  # [32]
    ang = pos[:, None] * inv_freq[None, :]          # [S, 32]
    cos = np.cos(ang).astype(np.float16)            # [S, 32]
    sin = np.sin(ang).astype(np.float16)

    # interleaved pair layout: partition p (within a head's 64) has freq p//2
    cos_rows = np.repeat(cos.T, 2, axis=0)          # [64, S]
    sin_rows = np.repeat(sin.T, 2, axis=0)
    sgn = np.where(np.arange(64) % 2 == 0, -1.0, 1.0).astype(np.float16)
    ssin_rows = sin_rows * sgn[:, None]
    crep = np.vstack([cos_rows, cos_rows])          # [128, 2048]
    ssign = np.vstack([ssin_rows, ssin_rows])

    # 0/1 triangle for the diagonal 128-col slab of each diag block:
    # column cc (local) valid iff cc >= p (key partition index)
    cc = np.arange(KCH)[None, :]
    ppi = np.arange(KCH)[:, None]
    tri = (cc >= ppi).astype(np.float16)            # [128, 128]
    mask01 = np.ascontiguousarray(
        np.broadcast_to(tri[:, None, :], (KCH, 2, KCH)))

    scale = 1.0 / math.sqrt(DK)
    in_maps = []
    for c in range(NCORES):
        hA, hB = HPC * c, HPC * c + 1
        wq = np.empty((3 * 128, D), dtype=np.float32)
        wq[0:64] = w_qkv[hA * DK:(hA + 1) * DK] * scale
        wq[64:128] = w_qkv[hB * DK:(hB + 1) * DK] * scale
        wq[128:192] = w_qkv[D + hA * DK:D + (hA + 1) * DK]
        wq[192:256] = w_qkv[D + hB * DK:D + (hB + 1) * DK]
        wq[256:320] = w_qkv[2 * D + hA * DK:2 * D + (hA + 1) * DK]
        wq[320:384] = w_qkv[2 * D + hB * DK:2 * D + (hB + 1) * DK]
        wqkvT = np.ascontiguousarray(wq.T).astype(np.float16)

        woTc = np.ascontiguousarray(
            w_o[:, hA * DK:(hB + 1) * DK].T).astype(np.float16)  # [128, 1024]

        in_maps.append({
            "xT": xT, "wqkvT": wqkvT, "woT": woTc,
            "crep": crep, "ssign": ssign, "mask01": mask01,
        })
    return in_maps


def _get_program():
    global _PROGRAM
    if _PROGRAM is None:
        _PROGRAM = _build_program()
    return _PROGRAM


def run_sharded(in_maps, **kwargs):
    nc = _get_program()
    return run_bass_kernel_spmd(nc, in_maps, core_ids=list(range(NCORES)),
                                **kwargs)


def kernel(x, token_positions, w_qkv, w_o):
    in_maps = _host_prep(x, token_positions, w_qkv, w_o)
    res = run_sharded(in_maps)
    acc = np.zeros((D, T), dtype=np.float64)
    for c in range(NCORES):
        acc += res.results[c]["yT"].astype(np.float32)
    y = acc.T.astype(np.float32).reshape(B, S, D)
    return y
